# revision 10
# baseline (speedup 1.0000x reference)
"""Trainium2 Bass kernel for per-element tiny MLPs.

Problem: N=4,000,000 independent 1->8->1 MLPs:
    y[i] = W2[i] @ relu(W1[i] * x[i] + b1[i]) + b2[i]

Memory-bound: 104 B/net in + 4 B/net out. Sharded over 8 NeuronCores by net
index (data parallel, no communication).

Device layout (per core, R = 500,000 nets):
  Tiles of T = 16*F2 nets. SBUF tile [128, F2]: partition p = j*16 + g holds
  hidden-unit j (j<8) of net-subgroup g (g<16); free index f is the net within
  the subgroup. Weights are host-pre-tiled so every weight DMA is a contiguous
  [128, F2] block.

Per tile t (k = t mod 8 within an 8-tile group):
  PE   : xb = Bbc.T @ xs_t      (0/1 matmul: broadcast x across j, exact fp32)
  DVE  : s = xb * W1
  DVE  : s = s + b1
  ACT  : s = relu(s)
  DVE  : s = s * W2
  PE   : ally += Bred8[k].T @ s (0/1 matmul: sums over j land in rows
                                 16k..16k+16 of a group-wide [128,F2] PSUM
                                 accumulator, zeros elsewhere — exact fp32)
Per group of 8 tiles:
  DVE  : yg = ally + b2g        (full-width add)
  DMA out yg [128, F2].
"""

import numpy as np
from contextlib import ExitStack

import concourse.bacc as bacc
import concourse.mybir as mybir
import concourse.tile as tile
from concourse.bass_utils import run_bass_kernel_spmd

F32 = mybir.dt.float32
AF = mybir.ActivationFunctionType
OP = mybir.AluOpType

N = 4_000_000
H = 8
GP = 16            # net subgroups per tile (partitions = H*GP = 128)
N_CORES = 8
R = N // N_CORES   # nets per core

# 30 full tiles of F2=1024 plus one ragged tile: 16*(30*1024 + 530) = 500,000
F2S = [1024] * 30 + [530]
GROUP_W = 1024
TPG = 8            # tiles per group (one PSUM accumulator / output buffer)
MM_CHUNK = 512     # max fp32 matmul free dim == one PSUM bank


def _groups(f2s):
    return [list(range(i, min(i + TPG, len(f2s)))) for i in range(0, len(f2s), TPG)]


def _chunks(f2):
    return [(i, min(MM_CHUNK, f2 - i)) for i in range(0, f2, MM_CHUNK)]


def _bbc():
    m = np.zeros((GP, 128), np.float32)
    for j in range(H):
        for g in range(GP):
            m[g, j * GP + g] = 1.0
    return m


def _bred8():
    # [128, TPG*128]: column block k holds Bred_k (maps (j,g) -> row 16k+g)
    m = np.zeros((128, TPG * 128), np.float32)
    for k in range(TPG):
        for j in range(H):
            for g in range(GP):
                m[j * GP + g, 128 * k + GP * k + g] = 1.0
    return m


def build_nc(f2s, group_w):
    groups = _groups(f2s)
    ng = len(groups)
    total = 128 * sum(f2s)
    rc = GP * sum(f2s)

    nc = bacc.Bacc("TRN2", target_bir_lowering=False, debug=False)

    w1 = nc.dram_tensor("w1", [total], F32, kind="ExternalInput")
    b1 = nc.dram_tensor("b1", [total], F32, kind="ExternalInput")
    w2 = nc.dram_tensor("w2", [total], F32, kind="ExternalInput")
    xs = nc.dram_tensor("xs", [rc], F32, kind="ExternalInput")
    b2g = nc.dram_tensor("b2g", [ng, 128, group_w], F32, kind="ExternalInput")
    bbc = nc.dram_tensor("bbc", [GP, 128], F32, kind="ExternalInput")
    bred = nc.dram_tensor("bred", [128, TPG * 128], F32, kind="ExternalInput")
    yg = nc.dram_tensor("yg", [ng, 128, group_w], F32, kind="ExternalOutput")

    with tile.TileContext(nc) as tc, ExitStack() as ctx:
        cpool = ctx.enter_context(tc.tile_pool(name="consts", bufs=1))
        wpool = ctx.enter_context(tc.tile_pool(name="w", bufs=3))
        spool = ctx.enter_context(tc.tile_pool(name="s", bufs=3))
        xpool = ctx.enter_context(tc.tile_pool(name="x", bufs=3))
        gpool = ctx.enter_context(tc.tile_pool(name="g", bufs=2))
        pxpool = ctx.enter_context(tc.tile_pool(name="px", bufs=2, space="PSUM"))
        pypool = ctx.enter_context(tc.tile_pool(name="py", bufs=2, space="PSUM"))

        bbc_s = cpool.tile([GP, 128], F32, tag="bbc")
        nc.sync.dma_start(bbc_s[:], bbc.ap())
        bred_s = cpool.tile([128, TPG * 128], F32, tag="bred")
        nc.sync.dma_start(bred_s[:], bred.ap())

        off = 0
        xoff = 0
        for gi, tl in enumerate(groups):
            b2g_s = gpool.tile([128, group_w], F32, tag="b2g")
            nc.sync.dma_start(b2g_s[:], b2g.ap()[gi])
            yg_s = gpool.tile([128, group_w], F32, tag="yg")
            ally = pypool.tile([128, group_w], F32, tag="ally")

            for k, t in enumerate(tl):
                f2 = f2s[t]
                blk = 128 * f2

                w1t = wpool.tile([128, f2], F32, tag="w1t")
                nc.sync.dma_start(
                    w1t[:], w1.ap()[off:off + blk].rearrange("(p f) -> p f", p=128)
                )
                b1t = wpool.tile([128, f2], F32, tag="b1t")
                nc.sync.dma_start(
                    b1t[:], b1.ap()[off:off + blk].rearrange("(p f) -> p f", p=128)
                )
                w2t = wpool.tile([128, f2], F32, tag="w2t")
                nc.sync.dma_start(
                    w2t[:], w2.ap()[off:off + blk].rearrange("(p f) -> p f", p=128)
                )
                off += blk

                xs_s = xpool.tile([GP, f2], F32, tag="xs")
                nc.sync.dma_start(
                    xs_s[:],
                    xs.ap()[xoff:xoff + GP * f2].rearrange("(g f) -> g f", g=GP),
                )
                xoff += GP * f2

                xb = pxpool.tile([128, f2], F32, tag="xb")
                for c0, cw in _chunks(f2):
                    nc.tensor.matmul(
                        xb[:, c0:c0 + cw],
                        bbc_s[:],
                        xs_s[:, c0:c0 + cw],
                        start=True,
                        stop=True,
                    )

                s = spool.tile([128, f2], F32, tag="s")
                nc.vector.tensor_tensor(s[:], xb[:], w1t[:], op=OP.mult)
                nc.vector.tensor_tensor(s[:], s[:], b1t[:], op=OP.add)
                nc.scalar.activation(s[:], s[:], AF.Relu)
                nc.vector.tensor_tensor(s[:], s[:], w2t[:], op=OP.mult)

                # accumulate this tile's j-sums into rows 16k..16k+16 of ally
                for c0, cw in _chunks(f2):
                    nc.tensor.matmul(
                        ally[:, c0:c0 + cw],
                        bred_s[:, 128 * k:128 * (k + 1)],
                        s[:, c0:c0 + cw],
                        start=(k == 0),
                        stop=(k == len(tl) - 1),
                    )

            nc.vector.tensor_tensor(yg_s[:], ally[:], b2g_s[:], op=OP.add)
            nc.scalar.dma_start(yg.ap()[gi], yg_s[:])

    nc.compile()
    return nc


# ---------------- host-side packing ----------------

def pack_w(wc, f2s):
    """wc [Rc, 8] -> flat [8*Rc] pre-tiled: per tile a [128, f2] block with
    partition p = j*16+g, free = f; net(t,g,f) = off_t + g*f2 + f."""
    parts = []
    o = 0
    for f2 in f2s:
        t = wc[o:o + GP * f2].reshape(GP, f2, H).transpose(2, 0, 1)
        parts.append(np.ascontiguousarray(t).reshape(-1))
        o += GP * f2
    return np.concatenate(parts)


def pack_g(v, f2s, group_w):
    """v [Rc] -> [ng, 128, group_w] group buffers (row block 16k..16k+16 of
    group gi holds tile t as [16, f2])."""
    groups = _groups(f2s)
    out = np.zeros((len(groups), 128, group_w), np.float32)
    o = 0
    for gi, tl in enumerate(groups):
        for k, t in enumerate(tl):
            f2 = f2s[t]
            out[gi, GP * k:GP * (k + 1), :f2] = v[o:o + GP * f2].reshape(GP, f2)
            o += GP * f2
    return out


def unpack_g(ygv, f2s, group_w):
    """[ng, 128, group_w] -> [Rc]"""
    groups = _groups(f2s)
    rc = GP * sum(f2s)
    out = np.empty(rc, np.float32)
    o = 0
    for gi, tl in enumerate(groups):
        for k, t in enumerate(tl):
            f2 = f2s[t]
            out[o:o + GP * f2] = ygv[gi, GP * k:GP * (k + 1), :f2].reshape(-1)
            o += GP * f2
    return out


# ---------------- entry point ----------------

_CACHE = {}


def _get_nc():
    if "nc" not in _CACHE:
        _CACHE["nc"] = build_nc(F2S, GROUP_W)
    return _CACHE["nc"]


def _make_in_maps(x, W1, b1, W2, b2):
    bbc, bred = _bbc(), _bred8()
    x = np.ascontiguousarray(x, np.float32)
    b2 = np.ascontiguousarray(b2, np.float32)
    in_maps = []
    for c in range(N_CORES):
        sl = slice(c * R, (c + 1) * R)
        in_maps.append({
            "w1": pack_w(np.asarray(W1[sl], np.float32), F2S),
            "b1": pack_w(np.asarray(b1[sl], np.float32), F2S),
            "w2": pack_w(np.asarray(W2[sl], np.float32), F2S),
            "xs": np.ascontiguousarray(x[sl, 0], np.float32),
            "b2g": pack_g(b2[sl, 0], F2S, GROUP_W),
            "bbc": bbc,
            "bred": bred,
        })
    return in_maps


def _run(x, W1, b1, W2, b2, **kw):
    nc = _get_nc()
    res = run_bass_kernel_spmd(nc, _make_in_maps(x, W1, b1, W2, b2),
                               core_ids=list(range(N_CORES)), **kw)
    y = np.empty((N, 1), np.float32)
    for c in range(N_CORES):
        y[c * R:(c + 1) * R, 0] = unpack_g(res.results[c]["yg"], F2S, GROUP_W)
    return y, res


def kernel(x, W1, b1, W2, b2):
    y, _ = _run(x, W1, b1, W2, b2)
    return y


# revision 13
# speedup vs baseline: 1.2313x; 1.2313x over previous
"""Trainium2 Bass kernel for per-element tiny MLPs.

Problem: N=4,000,000 independent 1->8->1 MLPs:
    y[i] = W2[i] @ relu(W1[i] * x[i] + b1[i]) + b2[i]

Memory-bound: 104 B/net in + 4 B/net out. Sharded over 8 NeuronCores by net
index (data parallel, no communication).

Device layout (per core, R = 500,000 nets):
  Tiles of T = 16*F2 nets. SBUF tile [128, F2]: partition p = j*16 + g holds
  hidden-unit j (j<8) of net-subgroup g (g<16); free index f is the net within
  the subgroup. Weights are host-pre-tiled so every weight DMA is a contiguous
  [128, F2] block.

Per tile t (k = t mod 8 within an 8-tile group):
  PE   : xb = Bbc.T @ xs_t      (0/1 matmul: broadcast x across j, exact fp32)
  DVE  : s = xb * W1
  DVE  : s = s + b1
  ACT  : s = relu(s)
  DVE  : s = s * W2
  PE   : ally += Bred8[k].T @ s (0/1 matmul: sums over j land in rows
                                 16k..16k+16 of a group-wide [128,F2] PSUM
                                 accumulator, zeros elsewhere — exact fp32)
Per group of 8 tiles:
  DVE  : yg = ally + b2g        (full-width add)
  DMA out yg [128, F2].
"""

import numpy as np
from contextlib import ExitStack

import concourse.bacc as bacc
import concourse.mybir as mybir
import concourse.tile as tile
from concourse.bass_utils import run_bass_kernel_spmd

F32 = mybir.dt.float32
AF = mybir.ActivationFunctionType
OP = mybir.AluOpType

N = 4_000_000
H = 8
GP = 16            # net subgroups per tile (partitions = H*GP = 128)
N_CORES = 8
R = N // N_CORES   # nets per core

# 30 full tiles of F2=1024 plus one ragged tile: 16*(30*1024 + 530) = 500,000
F2S = [1024] * 30 + [530]
GROUP_W = 1024
TPG = 8            # tiles per group (one PSUM accumulator / output buffer)
MM_CHUNK = 512     # max fp32 matmul free dim == one PSUM bank


def _groups(f2s):
    return [list(range(i, min(i + TPG, len(f2s)))) for i in range(0, len(f2s), TPG)]


def _chunks(f2):
    return [(i, min(MM_CHUNK, f2 - i)) for i in range(0, f2, MM_CHUNK)]


def _bbc():
    m = np.zeros((GP, 128), np.float32)
    for j in range(H):
        for g in range(GP):
            m[g, j * GP + g] = 1.0
    return m


def _bred8():
    # [128, TPG*128]: column block k holds Bred_k (maps (j,g) -> row 16k+g)
    m = np.zeros((128, TPG * 128), np.float32)
    for k in range(TPG):
        for j in range(H):
            for g in range(GP):
                m[j * GP + g, 128 * k + GP * k + g] = 1.0
    return m


def build_nc(f2s, group_w):
    groups = _groups(f2s)
    ng = len(groups)
    total = 128 * sum(f2s)
    rc = GP * sum(f2s)

    nc = bacc.Bacc("TRN2", target_bir_lowering=False, debug=False)

    w1 = nc.dram_tensor("w1", [total], F32, kind="ExternalInput")
    b1 = nc.dram_tensor("b1", [total], F32, kind="ExternalInput")
    w2 = nc.dram_tensor("w2", [total], F32, kind="ExternalInput")
    xs = nc.dram_tensor("xs", [rc], F32, kind="ExternalInput")
    b2g = nc.dram_tensor("b2g", [ng, 128, group_w], F32, kind="ExternalInput")
    bbc = nc.dram_tensor("bbc", [GP, 128], F32, kind="ExternalInput")
    bred = nc.dram_tensor("bred", [128, TPG * 128], F32, kind="ExternalInput")
    yg = nc.dram_tensor("yg", [ng, 128, group_w], F32, kind="ExternalOutput")

    with tile.TileContext(nc) as tc, ExitStack() as ctx:
        cpool = ctx.enter_context(tc.tile_pool(name="consts", bufs=1))
        wpool = ctx.enter_context(tc.tile_pool(name="w", bufs=5))
        spool = ctx.enter_context(tc.tile_pool(name="s", bufs=5))
        xpool = ctx.enter_context(tc.tile_pool(name="x", bufs=5))
        gpool = ctx.enter_context(tc.tile_pool(name="g", bufs=3))
        pxpool = ctx.enter_context(tc.tile_pool(name="px", bufs=2, space="PSUM"))
        pypool = ctx.enter_context(tc.tile_pool(name="py", bufs=2, space="PSUM"))

        bbc_s = cpool.tile([GP, 128], F32, tag="bbc")
        nc.sync.dma_start(bbc_s[:], bbc.ap())
        bred_s = cpool.tile([128, TPG * 128], F32, tag="bred")
        nc.sync.dma_start(bred_s[:], bred.ap())

        off = 0
        xoff = 0
        for gi, tl in enumerate(groups):
            b2g_s = gpool.tile([128, group_w], F32, tag="b2g")
            nc.sync.dma_start(b2g_s[:], b2g.ap()[gi])
            yg_s = gpool.tile([128, group_w], F32, tag="yg")
            ally = pypool.tile([128, group_w], F32, tag="ally")

            for k, t in enumerate(tl):
                f2 = f2s[t]
                blk = 128 * f2

                w1t = wpool.tile([128, f2], F32, tag="w1t")
                nc.sync.dma_start(
                    w1t[:], w1.ap()[off:off + blk].rearrange("(p f) -> p f", p=128)
                )
                b1t = wpool.tile([128, f2], F32, tag="b1t")
                nc.scalar.dma_start(
                    b1t[:], b1.ap()[off:off + blk].rearrange("(p f) -> p f", p=128)
                )
                w2t = wpool.tile([128, f2], F32, tag="w2t")
                nc.sync.dma_start(
                    w2t[:], w2.ap()[off:off + blk].rearrange("(p f) -> p f", p=128)
                )
                off += blk

                xs_s = xpool.tile([GP, f2], F32, tag="xs")
                nc.scalar.dma_start(
                    xs_s[:],
                    xs.ap()[xoff:xoff + GP * f2].rearrange("(g f) -> g f", g=GP),
                )
                xoff += GP * f2

                xb = pxpool.tile([128, f2], F32, tag="xb")
                for c0, cw in _chunks(f2):
                    nc.tensor.matmul(
                        xb[:, c0:c0 + cw],
                        bbc_s[:],
                        xs_s[:, c0:c0 + cw],
                        start=True,
                        stop=True,
                    )

                s = spool.tile([128, f2], F32, tag="s")
                nc.vector.tensor_tensor(s[:], xb[:], w1t[:], op=OP.mult)
                nc.vector.tensor_tensor(s[:], s[:], b1t[:], op=OP.add)
                nc.scalar.activation(s[:], s[:], AF.Relu)
                nc.vector.tensor_tensor(s[:], s[:], w2t[:], op=OP.mult)

                # accumulate this tile's j-sums into rows 16k..16k+16 of ally
                for c0, cw in _chunks(f2):
                    nc.tensor.matmul(
                        ally[:, c0:c0 + cw],
                        bred_s[:, 128 * k:128 * (k + 1)],
                        s[:, c0:c0 + cw],
                        start=(k == 0),
                        stop=(k == len(tl) - 1),
                    )

            nc.vector.tensor_tensor(yg_s[:], ally[:], b2g_s[:], op=OP.add)
            nc.scalar.dma_start(yg.ap()[gi], yg_s[:])

    nc.compile()
    return nc


# ---------------- host-side packing ----------------

def pack_w(wc, f2s):
    """wc [Rc, 8] -> flat [8*Rc] pre-tiled: per tile a [128, f2] block with
    partition p = j*16+g, free = f; net(t,g,f) = off_t + g*f2 + f."""
    parts = []
    o = 0
    for f2 in f2s:
        t = wc[o:o + GP * f2].reshape(GP, f2, H).transpose(2, 0, 1)
        parts.append(np.ascontiguousarray(t).reshape(-1))
        o += GP * f2
    return np.concatenate(parts)


def pack_g(v, f2s, group_w):
    """v [Rc] -> [ng, 128, group_w] group buffers (row block 16k..16k+16 of
    group gi holds tile t as [16, f2])."""
    groups = _groups(f2s)
    out = np.zeros((len(groups), 128, group_w), np.float32)
    o = 0
    for gi, tl in enumerate(groups):
        for k, t in enumerate(tl):
            f2 = f2s[t]
            out[gi, GP * k:GP * (k + 1), :f2] = v[o:o + GP * f2].reshape(GP, f2)
            o += GP * f2
    return out


def unpack_g(ygv, f2s, group_w):
    """[ng, 128, group_w] -> [Rc]"""
    groups = _groups(f2s)
    rc = GP * sum(f2s)
    out = np.empty(rc, np.float32)
    o = 0
    for gi, tl in enumerate(groups):
        for k, t in enumerate(tl):
            f2 = f2s[t]
            out[o:o + GP * f2] = ygv[gi, GP * k:GP * (k + 1), :f2].reshape(-1)
            o += GP * f2
    return out


# ---------------- entry point ----------------

_CACHE = {}


def _get_nc():
    if "nc" not in _CACHE:
        _CACHE["nc"] = build_nc(F2S, GROUP_W)
    return _CACHE["nc"]


def _make_in_maps(x, W1, b1, W2, b2):
    bbc, bred = _bbc(), _bred8()
    x = np.ascontiguousarray(x, np.float32)
    b2 = np.ascontiguousarray(b2, np.float32)
    in_maps = []
    for c in range(N_CORES):
        sl = slice(c * R, (c + 1) * R)
        in_maps.append({
            "w1": pack_w(np.asarray(W1[sl], np.float32), F2S),
            "b1": pack_w(np.asarray(b1[sl], np.float32), F2S),
            "w2": pack_w(np.asarray(W2[sl], np.float32), F2S),
            "xs": np.ascontiguousarray(x[sl, 0], np.float32),
            "b2g": pack_g(b2[sl, 0], F2S, GROUP_W),
            "bbc": bbc,
            "bred": bred,
        })
    return in_maps


def _run(x, W1, b1, W2, b2, **kw):
    nc = _get_nc()
    res = run_bass_kernel_spmd(nc, _make_in_maps(x, W1, b1, W2, b2),
                               core_ids=list(range(N_CORES)), **kw)
    y = np.empty((N, 1), np.float32)
    for c in range(N_CORES):
        y[c * R:(c + 1) * R, 0] = unpack_g(res.results[c]["yg"], F2S, GROUP_W)
    return y, res


def kernel(x, W1, b1, W2, b2):
    y, _ = _run(x, W1, b1, W2, b2)
    return y


# revision 14
# speedup vs baseline: 1.2432x; 1.0096x over previous
"""Trainium2 Bass kernel for per-element tiny MLPs.

Problem: N=4,000,000 independent 1->8->1 MLPs:
    y[i] = W2[i] @ relu(W1[i] * x[i] + b1[i]) + b2[i]

Memory-bound: 104 B/net in + 4 B/net out. Sharded over 8 NeuronCores by net
index (data parallel, no communication).

Device layout (per core, R=500,000 nets padded to R_PAD=128*3907): natural
interleaved layout — tile t covers 128*Fi nets; partition p holds nets
[base + p*Fi, base + (p+1)*Fi); the hidden dim j stays innermost in the free
dim, i.e. a weight tile is [128, Fi*8] and is a contiguous slice of the
natural [N, 8] array. No host-side repacking beyond pad+slice.

Per tile (everything in the free dim; no PE, no PSUM):
  DVE    : z = broadcast(x) * W1    (in0 has a step-0 inner AP dim - exact)
  DVE    : z = z + b1
  ACT    : z = relu(z)
  GPSIMD : u = z * W2
  DVE    : y = segmented_reduce_8(u)   (tensor_reduce axis=X on [128,Fi,8])
  DVE    : y = y + b2
"""

import numpy as np
from contextlib import ExitStack

import concourse.bacc as bacc
import concourse.mybir as mybir
import concourse.tile as tile
from concourse.bass_utils import run_bass_kernel_spmd

F32 = mybir.dt.float32
AF = mybir.ActivationFunctionType
OP = mybir.AluOpType
AX = mybir.AxisListType

N = 4_000_000
H = 8
N_CORES = 8
R = N // N_CORES            # 500,000 nets per core
FP = 3907                   # nets per partition (padded): 128*3907 = 500,096
R_PAD = 128 * FP
FIS = [384] * 10 + [67]     # per-tile nets-per-partition; sum == FP


def build_nc(fis):
    fp = sum(fis)
    rp = 128 * fp

    nc = bacc.Bacc("TRN2", target_bir_lowering=False, debug=False)

    w1 = nc.dram_tensor("w1", [rp, H], F32, kind="ExternalInput")
    b1 = nc.dram_tensor("b1", [rp, H], F32, kind="ExternalInput")
    w2 = nc.dram_tensor("w2", [rp, H], F32, kind="ExternalInput")
    xs = nc.dram_tensor("xs", [rp], F32, kind="ExternalInput")
    b2 = nc.dram_tensor("b2", [rp], F32, kind="ExternalInput")
    ys = nc.dram_tensor("ys", [rp], F32, kind="ExternalOutput")

    with tile.TileContext(nc) as tc, ExitStack() as ctx:
        wpool = ctx.enter_context(tc.tile_pool(name="w", bufs=4))
        vpool = ctx.enter_context(tc.tile_pool(name="v", bufs=4))

        nb = 0
        for fi in fis:
            nrows = 128 * fi
            wsl = lambda t: t.ap()[nb:nb + nrows, :].rearrange(
                "(p f) j -> p (f j)", p=128
            )
            vsl = lambda t: t.ap()[nb:nb + nrows].rearrange("(p f) -> p f", p=128)

            w1t = wpool.tile([128, fi * H], F32, tag="w1t")
            nc.sync.dma_start(w1t[:], wsl(w1))
            b1t = wpool.tile([128, fi * H], F32, tag="b1t")
            nc.scalar.dma_start(b1t[:], wsl(b1))
            w2t = wpool.tile([128, fi * H], F32, tag="w2t")
            nc.sync.dma_start(w2t[:], wsl(w2))
            xt = vpool.tile([128, fi], F32, tag="xt")
            nc.scalar.dma_start(xt[:], vsl(xs))
            b2t = vpool.tile([128, fi], F32, tag="b2t")
            nc.scalar.dma_start(b2t[:], vsl(b2))

            w1t3 = w1t[:].rearrange("p (f j) -> p f j", j=H)
            w2t3 = w2t[:].rearrange("p (f j) -> p f j", j=H)
            xb = xt[:].broadcast_to([128, fi, H])

            nc.vector.tensor_tensor(w1t3, xb, w1t3, op=OP.mult)
            nc.vector.tensor_tensor(w1t[:], w1t[:], b1t[:], op=OP.add)
            nc.scalar.activation(w1t[:], w1t[:], AF.Relu)
            nc.gpsimd.tensor_tensor(w2t[:], w1t[:], w2t[:], op=OP.mult)

            yt = vpool.tile([128, fi], F32, tag="yt")
            nc.vector.tensor_reduce(yt[:], w2t3, axis=AX.X, op=OP.add)
            nc.vector.tensor_tensor(yt[:], yt[:], b2t[:], op=OP.add)

            nc.sync.dma_start(vsl(ys), yt[:])
            nb += nrows

    nc.compile()
    return nc


# ---------------- entry point ----------------

_CACHE = {}


def _get_nc():
    if "nc" not in _CACHE:
        _CACHE["nc"] = build_nc(FIS)
    return _CACHE["nc"]


def _pad2(a):
    out = np.zeros((R_PAD, H), np.float32)
    out[:R] = a
    return out


def _pad1(a):
    out = np.zeros(R_PAD, np.float32)
    out[:R] = a
    return out


def _make_in_maps(x, W1, b1, W2, b2):
    x = np.ascontiguousarray(x, np.float32)
    b2 = np.ascontiguousarray(b2, np.float32)
    in_maps = []
    for c in range(N_CORES):
        sl = slice(c * R, (c + 1) * R)
        in_maps.append({
            "w1": _pad2(np.asarray(W1[sl], np.float32)),
            "b1": _pad2(np.asarray(b1[sl], np.float32)),
            "w2": _pad2(np.asarray(W2[sl], np.float32)),
            "xs": _pad1(x[sl, 0]),
            "b2": _pad1(b2[sl, 0]),
        })
    return in_maps


def _run(x, W1, b1, W2, b2, **kw):
    nc = _get_nc()
    res = run_bass_kernel_spmd(nc, _make_in_maps(x, W1, b1, W2, b2),
                               core_ids=list(range(N_CORES)), **kw)
    y = np.empty((N, 1), np.float32)
    for c in range(N_CORES):
        y[c * R:(c + 1) * R, 0] = res.results[c]["ys"].reshape(-1)[:R]
    return y, res


def kernel(x, W1, b1, W2, b2):
    y, _ = _run(x, W1, b1, W2, b2)
    return y


# revision 18
# speedup vs baseline: 1.2612x; 1.0145x over previous
"""Trainium2 Bass kernel for per-element tiny MLPs.

Problem: N=4,000,000 independent 1->8->1 MLPs:
    y[i] = W2[i] @ relu(W1[i] * x[i] + b1[i]) + b2[i]

Memory-bound: 104 B/net in + 4 B/net out. Sharded over 8 NeuronCores by net
index (data parallel, no communication).

Device layout (per core, R=500,000 nets padded to R_PAD=128*3907): natural
interleaved layout — tile t covers 128*Fi nets; partition p holds nets
[base + p*Fi, base + (p+1)*Fi); the hidden dim j stays innermost in the free
dim, i.e. a weight tile is [128, Fi*8] and is a contiguous slice of the
natural [N, 8] array. No host-side repacking beyond pad+slice.

Per tile (everything in the free dim; no PE, no PSUM):
  DVE    : z = broadcast(x) * W1    (in0 has a step-0 inner AP dim - exact)
  DVE    : z = z + b1
  ACT    : z = relu(z)
  GPSIMD : u = z * W2
  DVE    : y = segmented_reduce_8(u)   (tensor_reduce axis=X on [128,Fi,8])
  DVE    : y = y + b2
"""

import numpy as np
from contextlib import ExitStack

import concourse.bacc as bacc
import concourse.mybir as mybir
import concourse.tile as tile
from concourse.bass_utils import run_bass_kernel_spmd

F32 = mybir.dt.float32
AF = mybir.ActivationFunctionType
OP = mybir.AluOpType
AX = mybir.AxisListType

N = 4_000_000
H = 8
N_CORES = 8
R = N // N_CORES            # 500,000 nets per core
FP = 3907                   # nets per partition (padded): 128*3907 = 500,096
R_PAD = 128 * FP
FIS = [288] * 13 + [163]    # per-tile nets-per-partition; sum == FP


def build_nc(fis):
    fp = sum(fis)
    rp = 128 * fp

    nc = bacc.Bacc("TRN2", target_bir_lowering=False, debug=False)

    w1 = nc.dram_tensor("w1", [rp, H], F32, kind="ExternalInput")
    b1 = nc.dram_tensor("b1", [rp, H], F32, kind="ExternalInput")
    w2 = nc.dram_tensor("w2", [rp, H], F32, kind="ExternalInput")
    xs = nc.dram_tensor("xs", [rp], F32, kind="ExternalInput")
    b2 = nc.dram_tensor("b2", [rp], F32, kind="ExternalInput")
    ys = nc.dram_tensor("ys", [rp], F32, kind="ExternalOutput")

    with tile.TileContext(nc) as tc, ExitStack() as ctx:
        wpool = ctx.enter_context(tc.tile_pool(name="w", bufs=3))
        zpool = ctx.enter_context(tc.tile_pool(name="z", bufs=2))
        vpool = ctx.enter_context(tc.tile_pool(name="v", bufs=4))

        nb = 0
        for fi in fis:
            nrows = 128 * fi
            wsl = lambda t: t.ap()[nb:nb + nrows, :].rearrange(
                "(p f) j -> p (f j)", p=128
            )
            vsl = lambda t: t.ap()[nb:nb + nrows].rearrange("(p f) -> p f", p=128)

            w1t = wpool.tile([128, fi * H], F32, tag="w1t")
            nc.sync.dma_start(w1t[:], wsl(w1))
            b1t = wpool.tile([128, fi * H], F32, tag="b1t")
            nc.scalar.dma_start(b1t[:], wsl(b1))
            w2t = wpool.tile([128, fi * H], F32, tag="w2t")
            nc.sync.dma_start(w2t[:], wsl(w2))
            xt = vpool.tile([128, fi], F32, tag="xt")
            nc.scalar.dma_start(xt[:], vsl(xs))
            b2t = vpool.tile([128, fi], F32, tag="b2t")
            nc.scalar.dma_start(b2t[:], vsl(b2))

            w1t3 = w1t[:].rearrange("p (f j) -> p f j", j=H)
            xb = xt[:].broadcast_to([128, fi, H])

            za = zpool.tile([128, fi * H], F32, tag="za")
            zb = zpool.tile([128, fi * H], F32, tag="zb")
            zc = zpool.tile([128, fi * H], F32, tag="zc")
            zd = zpool.tile([128, fi * H], F32, tag="zd")

            nc.vector.tensor_tensor(
                za[:].rearrange("p (f j) -> p f j", j=H), xb, w1t3, op=OP.mult
            )
            nc.vector.tensor_tensor(zb[:], za[:], b1t[:], op=OP.add)
            nc.scalar.activation(zc[:], zb[:], AF.Relu)
            nc.gpsimd.tensor_tensor(zd[:], zc[:], w2t[:], op=OP.mult)

            yt = vpool.tile([128, fi], F32, tag="yt")
            nc.vector.tensor_reduce(
                yt[:], zd[:].rearrange("p (f j) -> p f j", j=H), axis=AX.X, op=OP.add
            )
            yo = vpool.tile([128, fi], F32, tag="yo")
            nc.vector.tensor_tensor(yo[:], yt[:], b2t[:], op=OP.add)

            nc.sync.dma_start(vsl(ys), yo[:])
            nb += nrows

    nc.compile()
    return nc


# ---------------- entry point ----------------

_CACHE = {}


def _get_nc():
    if "nc" not in _CACHE:
        _CACHE["nc"] = build_nc(FIS)
    return _CACHE["nc"]


def _pad2(a):
    out = np.zeros((R_PAD, H), np.float32)
    out[:R] = a
    return out


def _pad1(a):
    out = np.zeros(R_PAD, np.float32)
    out[:R] = a
    return out


def _make_in_maps(x, W1, b1, W2, b2):
    x = np.ascontiguousarray(x, np.float32)
    b2 = np.ascontiguousarray(b2, np.float32)
    in_maps = []
    for c in range(N_CORES):
        sl = slice(c * R, (c + 1) * R)
        in_maps.append({
            "w1": _pad2(np.asarray(W1[sl], np.float32)),
            "b1": _pad2(np.asarray(b1[sl], np.float32)),
            "w2": _pad2(np.asarray(W2[sl], np.float32)),
            "xs": _pad1(x[sl, 0]),
            "b2": _pad1(b2[sl, 0]),
        })
    return in_maps


def _run(x, W1, b1, W2, b2, **kw):
    nc = _get_nc()
    res = run_bass_kernel_spmd(nc, _make_in_maps(x, W1, b1, W2, b2),
                               core_ids=list(range(N_CORES)), **kw)
    y = np.empty((N, 1), np.float32)
    for c in range(N_CORES):
        y[c * R:(c + 1) * R, 0] = res.results[c]["ys"].reshape(-1)[:R]
    return y, res


def kernel(x, W1, b1, W2, b2):
    y, _ = _run(x, W1, b1, W2, b2)
    return y
